# revision 9
# baseline (speedup 1.0000x reference)
"""Trainium2 Bass kernel for a segmented tensor-product contraction.

Computation (per batch row z, channel u, segments of width U=128):
  out[z, so, u] += c_p * x0[i0[z], s0_p, u] * prod_k x1[z, sk_p, u]
for 256 paths of degree 1..3 over S=16 segments.

Design (measured 142.4us on 8 NeuronCores, data-parallel over z):
  - On-chip layout [u=128 partitions, z=512 free]; every elementwise op
    is a [128, 512] bf16 DVE instruction (2x perf mode).
  - x0 row gather on the TensorEngine via host-built one-hot matrices
    (keeps input DMA at 2.3MB/core so compute starts ~7us earlier than
    shipping the gathered 2MB table).
  - Joint factorization of all paths into shared sg(s0,s)=x0g[s0]*x1[s]
    and pair(a,b)=x1[a]*x1[b] products (merged multi-column builds);
    squares x1[s]^2 go to the otherwise-idle ScalarEngine.
  - Per-path term = one DVE multiply; merged final chunks accept any
    constant slot stride (not just 0/1), up to 5 wide, trimming ~45
    DVE instruction issues (~150ns fixed cost each).  The mm FIFO
    drains progressively over the last few event clusters so the PE
    finishes with (not after) the DVE.
  - Coefficient applied by the
    TensorEngine via diag(c_p) stationary matmuls accumulating into one
    PSUM bank per output segment (exact f32 adds). Weight slabs are
    prefetched one slab ahead.
  - Matmul emission runs through a 16-deep software FIFO so the PE sees
    dense back-to-back work (keeps it at the 2.4GHz p-state; a stalling
    PE throttles to 1.2GHz).
  - Per-so PSUM evacuation (ACT copy + DMA) fires immediately after the
    segment's last matmul to keep the tail short.
  - GPSIMD is deliberately idle: it shares SBUF ports with the DVE, and
    measured Pool ops slow concurrent DVE work ~2-5x.
"""

import os
from collections import defaultdict

import numpy as np

U = 128
S = 16
NELEM = 64
Z = 4096
NCORES = 8
ZS = Z // NCORES  # 512 rows per core

LAST_EXEC_NS = None
LAST_RESULTS = None

SLAB = 16  # coefficient-diagonal matrices per DMA slab


def _parse_paths(idxs, coeffs):
    paths = []  # (degree, x1segs_sorted, s0, so, coeff)
    for idx, cf in zip(idxs, coeffs):
        d = idx.shape[1] - 2
        for r, c in zip(idx, cf):
            r = [int(v) for v in r]
            paths.append((d, tuple(sorted(r[:d])), r[d], r[d + 1], float(c)))
    return paths


def _options(p):
    d, segs, s0, so, c = p
    if d == 1:
        k = ("sg", (s0, segs[0]))
        return [(frozenset([k]), (k, None))]
    if d == 2:
        a, b = segs
        return [
            (frozenset([("sg", (s0, b))]), (("x1", a), ("sg", (s0, b)))),
            (frozenset([("sg", (s0, a))]), (("x1", b), ("sg", (s0, a)))),
            (frozenset([("pair", (a, b))]), (("pair", (a, b)), ("x0g", s0))),
        ]
    a, b, cc = segs
    return [
        (
            frozenset([("pair", (a, b)), ("sg", (s0, cc))]),
            (("pair", (a, b)), ("sg", (s0, cc))),
        ),
        (
            frozenset([("pair", (a, cc)), ("sg", (s0, b))]),
            (("pair", (a, cc)), ("sg", (s0, b))),
        ),
        (
            frozenset([("pair", (b, cc)), ("sg", (s0, a))]),
            (("pair", (b, cc)), ("sg", (s0, a))),
        ),
    ]


def _optimize_group(gpaths, n_sweeps=4):
    choices = [0] * len(gpaths)
    opts = [_options(p) for p in gpaths]
    for _ in range(n_sweeps):
        counts = defaultdict(int)
        for i, p in enumerate(gpaths):
            for k in opts[i][choices[i]][0]:
                counts[k] += 1
        changed = False
        for i, p in enumerate(gpaths):
            best, best_cost = choices[i], None
            for j, (prods, _) in enumerate(opts[i]):
                cost = 0.0
                for k in prods:
                    others = counts[k] - (1 if k in opts[i][choices[i]][0] else 0)
                    cost += 1.0 / (1 + others)
                if best_cost is None or cost < best_cost - 1e-9:
                    best, best_cost = j, cost
            if best != choices[i]:
                for k in opts[i][choices[i]][0]:
                    counts[k] -= 1
                for k in opts[i][best][0]:
                    counts[k] += 1
                choices[i] = best
                changed = True
        if not changed:
            break
    products = set()
    forms = []
    for i, p in enumerate(gpaths):
        prods, form = opts[i][choices[i]]
        products |= prods
        forms.append(form)
    return products, forms


def _plan_merges(products):
    """Pack product builds into merged runs.

    Returns (slot_of, builds, n_slots) where builds is a list of
    ('sg_run', s0, s_lo, n, slot_lo) or ('pair_run', delta, a_lo, n,
    slot_lo)."""
    slot_of = {}
    builds = []
    next_slot = 0
    sgs = defaultdict(list)
    prs = defaultdict(list)
    for k in products:
        if k[0] == "sg":
            sgs[k[1][0]].append(k[1][1])
        else:
            a, b = k[1]
            prs[b - a].append(a)
    squares = sorted(prs.pop(0, []))
    for delta in sorted(prs):
        aa = sorted(prs[delta])
        run = [aa[0]]
        for a in aa[1:] + [None]:
            if a is not None and a == run[-1] + 1:
                run.append(a)
            else:
                builds.append(("pair_run", delta, run[0], len(run), next_slot))
                for i, ra in enumerate(run):
                    slot_of[("pair", (ra, ra + delta))] = next_slot + i
                next_slot += len(run)
                if a is not None:
                    run = [a]
    for s0 in sorted(sgs):
        ss = sorted(sgs[s0])
        run = [ss[0]]
        for s in ss[1:] + [None]:
            if s is not None and s == run[-1] + 1:
                run.append(s)
            else:
                builds.append(("sg_run", s0, run[0], len(run), next_slot))
                for i, rs in enumerate(run):
                    slot_of[("sg", (s0, rs))] = next_slot + i
                next_slot += len(run)
                if s is not None:
                    run = [s]
    return slot_of, builds, next_slot, squares


def _plan_sg_only(products):
    """_plan_merges for sg products only (pairs handled separately)."""
    slot_of = {}
    builds = []
    next_slot = 0
    sgs = defaultdict(list)
    for k in products:
        if k[0] == "sg":
            sgs[k[1][0]].append(k[1][1])
    for s0 in sorted(sgs):
        ss = sorted(sgs[s0])
        run = [ss[0]]
        for s in ss[1:] + [None]:
            if s is not None and s == run[-1] + 1:
                run.append(s)
            else:
                builds.append(("sg_run", s0, run[0], len(run), next_slot))
                for i, rs in enumerate(run):
                    slot_of[("sg", (s0, rs))] = next_slot + i
                next_slot += len(run)
                if s is not None:
                    run = [s]
    return slot_of, builds, next_slot


def _build_plan(idxs, coeffs):
    """Clustered plan: per group, emission = sequence of events
    ('build_sg', s0, lo, n, slot), ('build_pair', a, b, slot),
    ('mm', path_idx), ('finals', [path_idx...]).

    Pair slots are assigned in final-use order so that per-s0 final runs
    have affine (delta 0/1) operand patterns and merge into single DVE
    instructions."""
    paths = _parse_paths(idxs, coeffs)
    products, forms = _optimize_group(paths, n_sweeps=6)
    part_a = list(range(8))
    part_b = list(range(8, 16))

    all_sq = sorted(
        set(k[1][0] for k in products if k[0] == "pair" and k[1][0] == k[1][1])
    )
    sq_keys = set(("pair", (s, s)) for s in all_sq)

    use_a, use_b = set(), set()
    for p, form in zip(paths, forms):
        tgt = use_a if p[3] in part_a else use_b
        for r in form:
            if r and r[0] in ("sg", "pair") and r not in sq_keys:
                tgt.add(r)
    shared = use_a & use_b
    uniq = {0: use_a - shared, 1: use_b - shared}

    # sg slot regions: shared first, then per-group unique overlay
    slot_sh, builds_sh_sg, ns_sh = _plan_sg_only(shared)
    slot_a, builds_a_sg, ns_a = _plan_sg_only(uniq[0])
    slot_b, builds_b_sg, ns_b = _plan_sg_only(uniq[1])

    n_pair_sh = sum(1 for k in shared if k[0] == "pair")
    n_pair_u = {
        0: sum(1 for k in uniq[0] if k[0] == "pair"),
        1: sum(1 for k in uniq[1] if k[0] == "pair"),
    }
    # layout: [sg_sh | pair_sh | overlay(sg_u + pair_u) | squares]
    pair_sh_base = ns_sh
    over_base = ns_sh + n_pair_sh
    n_over = max(ns_a + n_pair_u[0], ns_b + n_pair_u[1])
    sq_base = over_base + n_over
    n_slots = sq_base + len(all_sq)
    sq_slot = {s: sq_base + i for i, s in enumerate(all_sq)}

    def shift(slot, delta):
        return {k: v + delta for k, v in slot.items()}

    slot_a = shift(slot_a, over_base)
    slot_b = shift(slot_b, over_base)
    builds_a_sg = [(k, s0, lo, n, sl + over_base) for k, s0, lo, n, sl in builds_a_sg]
    builds_b_sg = [(k, s0, lo, n, sl + over_base) for k, s0, lo, n, sl in builds_b_sg]

    # pair slots: merged (delta, a)-runs per region, built lazily at first use
    def plan_pair_runs(keys, base):
        prs = defaultdict(list)
        for k in keys:
            if k[0] == "pair":
                a, b = k[1]
                prs[b - a].append(a)
        slot, runs = {}, []
        nxt = base
        for delta in sorted(prs):
            aa = sorted(prs[delta])
            run = [aa[0]]
            for a in aa[1:] + [None]:
                if a is not None and a == run[-1] + 1:
                    run.append(a)
                else:
                    runs.append((delta, run[0], len(run), nxt))
                    for i, ra in enumerate(run):
                        slot[("pair", (ra, ra + delta))] = nxt + i
                    nxt += len(run)
                    if a is not None:
                        run = [a]
        return slot, runs

    pr_slot_sh, pr_runs_sh = plan_pair_runs(shared, pair_sh_base)
    pr_slot_a, pr_runs_a = plan_pair_runs(uniq[0], over_base + ns_a)
    pr_slot_b, pr_runs_b = plan_pair_runs(uniq[1], over_base + ns_b)
    pair_slot = {**pr_slot_sh, **pr_slot_a, **pr_slot_b}
    emitted_pair_runs = set()
    run_of_pair = {}
    for runs in (pr_runs_sh, pr_runs_a, pr_runs_b):
        for r in runs:
            delta, lo, n, sl = r
            for i in range(n):
                run_of_pair[("pair", (lo + i, lo + i + delta))] = r

    groups = []
    for gi, (sos, sg_builds, sg_slots) in enumerate(
        (
            (part_a, builds_sh_sg + builds_a_sg, {**slot_sh, **slot_a}),
            (part_b, builds_b_sg, {**slot_sh, **slot_b}),
        )
    ):
        slot_of = dict(sg_slots)
        for s in all_sq:
            slot_of[("pair", (s, s))] = sq_slot[s]
        gidx = [i for i, p in enumerate(paths) if p[3] in sos]
        path_ops = {}
        for i in gidx:
            path_ops[i] = (
                paths[i][0],
                forms[i][0],
                forms[i][1],
                paths[i][4],
                paths[i][3],
            )

        # classify paths by their sg-side (in1) cluster
        by_s0 = defaultdict(list)  # s0 -> [(slot, path_idx, kind)]
        d2x0g = defaultdict(list)  # s0 -> [path_idx] (pair (x) x0g form)
        d1s = defaultdict(list)
        for i in gidx:
            d, r1, r2, c, so = path_ops[i]
            if d == 1:
                d1s[r1[1][0]].append(i)
            elif r2 and r2[0] == "sg":
                by_s0[r2[1][0]].append(i)
            elif r2 and r2[0] == "x0g":
                d2x0g[r2[1]].append(i)
            else:
                # squares-based d3: r2 could be sg; r1 pair-square handled
                by_s0[r2[1][0] if r2 and r2[0] == "sg" else -1].append(i)

        events = []
        sg_build_of = {}
        for b in sg_builds:
            sg_build_of[b[1]] = sg_build_of.get(b[1], []) + [b]
        emitted_sg = set()

        def ensure_pair(key):
            r = run_of_pair[key]
            if id(r) not in emitted_pair_runs:
                emitted_pair_runs.add(id(r))
                events.append(("build_pair_run",) + r)
            return pair_slot[key], False

        def final_runs(plist, in1_key):
            """Emit pair-run builds on demand; chunk finals where operand
            slot deltas are constant in {0,1}."""

            def in1_slot(i):
                r2 = path_ops[i][2]
                if r2[0] == "sg":
                    return slot_of[r2]
                return -1

            plist = sorted(plist, key=in1_slot)
            for i in plist:
                r1 = path_ops[i][1]
                if r1[0] == "pair" and r1 not in slot_of:
                    sl, _ = ensure_pair(r1)
                    slot_of[r1] = sl
            cur = []

            def sl_of(i, which):
                r = path_ops[i][which]
                if r[0] in ("sg", "pair"):
                    return (r[0] == "pair", slot_of[r])
                if r[0] == "x1":
                    return ("x1", r[1])
                return ("x0g", -1)

            for i in plist:
                ok = bool(cur)
                if ok:
                    k1, p1 = sl_of(i, 1), sl_of(cur[-1], 1)
                    k2, p2 = sl_of(i, 2), sl_of(cur[-1], 2)
                    ok = (
                        k1[0] == p1[0]
                        and k2[0] == p2[0]
                        and abs(k1[1] - p1[1]) <= 8
                        and abs(k2[1] - p2[1]) <= 8
                    )
                    if ok and len(cur) >= 2:
                        q1, q2 = sl_of(cur[-2], 1), sl_of(cur[-2], 2)
                        ok = (k1[1] - p1[1]) == (p1[1] - q1[1]) and (
                            k2[1] - p2[1]
                        ) == (p2[1] - q2[1])
                if ok and len(cur) < 5:
                    cur.append(i)
                else:
                    if cur:
                        events.append(("finals", cur))
                    cur = [i]
            if cur:
                events.append(("finals", cur))

        # cluster emission: s0 order by shared-first then slot order
        s0_list = sorted(
            set(list(by_s0.keys()) + list(d1s.keys()) + list(d2x0g.keys())),
            key=lambda s0: min(
                [slot_of[k] for k in slot_of if k[0] == "sg" and k[1][0] == s0]
                + [10**6]
            ),
        )
        for s0 in s0_list:
            for b in sg_build_of.get(s0, []):
                if id(b) not in emitted_sg:
                    emitted_sg.add(id(b))
                    events.append(("build_sg",) + b[1:])
            for i in d1s.get(s0, []):
                events.append(("mm", i))
            if s0 in by_s0:
                # split: d3 (pair in0) first, then d2 (x1 in0)
                d3l = [i for i in by_s0[s0] if path_ops[i][1][0] == "pair"]
                d2l = [i for i in by_s0[s0] if path_ops[i][1][0] == "x1"]
                if d3l:
                    final_runs(d3l, None)
                if d2l:
                    final_runs(d2l, None)
            if s0 in d2x0g:
                final_runs(d2x0g[s0], None)
        # any remaining sg builds (unused s0s) — shouldn't happen
        for s0, bs in sg_build_of.items():
            for b in bs:
                if id(b) not in emitted_sg:
                    emitted_sg.add(id(b))
                    events.append(("build_sg",) + b[1:])

        if gi == 0:
            # natural x1 first-use order of this event list
            nat = []
            for ev in events:
                segs = ()
                if ev[0] == "build_sg":
                    segs = range(ev[2], ev[2] + ev[3])
                elif ev[0] == "build_pair_run":
                    _, delta, lo, n, _sl = ev
                    segs = [lo + i for i in range(n)] + [
                        lo + i + delta for i in range(n)
                    ]
                for s in segs:
                    if s not in nat:
                        nat.append(s)
            rank = {s: k for k, s in enumerate(nat)}
            # hoist pair runs fully contained in the first 4 DMA'd segments:
            # they give the DVE x1-only work during the x0 gather latency
            hoist, rest = [], []
            for ev in events:
                if (
                    ev[0] == "build_pair_run"
                    and len(hoist) < 3
                    and all(
                        rank.get(s, 99) < 4
                        for s in list(range(ev[2], ev[2] + ev[3]))
                        + [ev[2] + i + ev[1] for i in range(ev[3])]
                    )
                ):
                    hoist.append(ev)
                else:
                    rest.append(ev)
            events = hoist + rest
        groups.append(
            dict(
                sos=sos,
                events=events,
                slot_of=dict(slot_of),
                n_slots=n_slots,
                path_ops=path_ops,
            )
        )
    return groups, all_sq, paths, forms


def _assign_pool(groups, pool_cols_budget):
    """Mark build runs for the GPSIMD engine up to the column budget.
    Longest runs first (amortize the Q7 launch), pair runs preferred
    (no x0g dependency). Returns per-group list of bools (pool?)."""
    allb = []
    for gi, g in enumerate(groups):
        for bi, b in enumerate(g["builds"]):
            allb.append((b[3], b[0] == "pair_run", gi, bi))
    allb.sort(key=lambda t: (-t[0], not t[1]))
    flags = {gi: [False] * len(g["builds"]) for gi, g in enumerate(groups)}
    cols = 0
    for n, ispair, gi, bi in allb:
        if cols >= pool_cols_budget:
            break
        flags[gi][bi] = True
        cols += n
    return flags


def _build_bass(groups, all_sq, pool_flags, ring_w, pipe_depth=0, stage_max=0):
    import bass_rust as _br
    import concourse.bacc as bacc
    import concourse.mybir as mybir
    from concourse.tile import TileContext

    dt = mybir.dt.bfloat16
    MULT = mybir.AluOpType.mult

    nc = bacc.Bacc("TRN2", debug=False)

    n_paths_total = sum(len(g["path_ops"]) for g in groups)
    n_slabs = (n_paths_total + SLAB - 1) // SLAB

    x1t_d = nc.dram_tensor("x1t", [S * U, ZS], dt, kind="ExternalInput")
    x0_d = nc.dram_tensor("x0w", [NELEM, S * U], dt, kind="ExternalInput")
    oh_d = nc.dram_tensor("oh", [NELEM, ZS], dt, kind="ExternalInput")
    cd_d = nc.dram_tensor("cdiag", [n_slabs * SLAB * U, U], dt, kind="ExternalInput")
    out_d = nc.dram_tensor("outt", [S * U, ZS], dt, kind="ExternalOutput")

    max_slots = max(g["n_slots"] for g in groups)
    coeff_order = []

    with TileContext(nc) as tc:
        with tc.tile_pool(name="persist", bufs=1) as persist, tc.tile_pool(
            name="ring", bufs=7
        ) as ring_pool, tc.tile_pool(name="slab", bufs=2) as slab_pool, tc.tile_pool(
            name="evac", bufs=3
        ) as evac_pool:
            x1t = persist.tile([U, S * ZS], dt, tag="x1t")
            x0g = persist.tile([U, S * ZS], dt, tag="x0g")
            prod = persist.tile([U, max_slots * ZS], dt, tag="prod")
            x0_sb = persist.tile([NELEM, S * U], dt, tag="x0w")
            oh_sb = persist.tile([NELEM, ZS], dt, tag="oh")

            def seg(t, s):
                return t[:, s * ZS : (s + 1) * ZS]

            def span(t, lo, n):
                return t[:, lo * ZS : (lo + n) * ZS]

            # ---- input DMAs: chunks ordered by first use (event scan)
            x1_order, x0_order = [], []

            def _want(lst, s):
                if s not in lst:
                    lst.append(s)

            for g in groups:
                for ev in g["events"]:
                    if ev[0] == "build_sg":
                        _, s0, lo, n, _sl = ev
                        _want(x0_order, s0)
                        for i in range(n):
                            _want(x1_order, lo + i)
                    elif ev[0] == "build_pair_run":
                        _, delta, lo, n, _sl = ev
                        for i in range(n):
                            _want(x1_order, lo + i)
                            _want(x1_order, lo + i + delta)
            for s in range(S):
                _want(x1_order, s)
                _want(x0_order, s)

            CH = 2

            def chunk_rank(c):
                return min(x1_order.index(s) for s in range(c * CH, (c + 1) * CH))

            # alternate x1t chunks across two DMA queues: the first DVE
            # op waits for ALL writers of the x1t tile (tile-granular
            # deps), so halving the serialized transfer time pulls the
            # whole DVE window ~5us earlier
            x1_queues = [nc.sync, nc.gpsimd]
            for qi, c in enumerate(sorted(range(S // CH), key=chunk_rank)):
                lo = c * CH
                x1_queues[qi % 2].dma_start(
                    out=span(x1t, lo, CH).rearrange("p (s z) -> p s z", s=CH),
                    in_=x1t_d[lo * U : (lo + CH) * U, :]
                    .rearrange("(s u) z -> u s z", u=U),
                )
            nc.scalar.dma_start(out=oh_sb[:], in_=oh_d[:])
            nc.scalar.dma_start(out=x0_sb[:], in_=x0_d[:])
            # device-side gather: x0g[s] = x0[:, s]^T @ onehot, PE + ACT
            with tc.tile_pool(name="gpsum", bufs=4, space="PSUM") as gpsum:
                for s in x0_order:
                    pt = gpsum.tile([U, ZS], mybir.dt.float32, tag="gps")
                    nc.tensor.matmul(
                        pt[:],
                        x0_sb[:, s * U : (s + 1) * U],
                        oh_sb[:],
                        start=True,
                        stop=True,
                    )
                    nc.scalar.copy(out=seg(x0g, s), in_=pt[:])

            # global square products on ACT
            if all_sq:
                sq_base = groups[0]["n_slots"] - len(all_sq)
                run = [all_sq[0]]
                ri = 0
                for s in list(all_sq[1:]) + [None]:
                    if s is not None and s == run[-1] + 1:
                        run.append(s)
                    else:
                        nc.scalar.activation(
                            span(prod, sq_base + ri, len(run)),
                            span(x1t, run[0], len(run)),
                            mybir.ActivationFunctionType.Square,
                        )
                        ri += len(run)
                        if s is not None:
                            run = [s]

            slab_state = {}

            from concourse.ap import AP as _AP

            for gi, g in enumerate(groups):
                sos, events, slot_of, path_ops = (
                    g["sos"],
                    g["events"],
                    g["slot_of"],
                    g["path_ops"],
                )

                # mm order for first/last_for_so
                mm_order = []
                for ev in events:
                    if ev[0] == "mm":
                        mm_order.append(ev[1])
                    elif ev[0] == "finals":
                        mm_order.extend(ev[1])
                first_for_so = {}
                last_for_so = {}
                for i in mm_order:
                    so = path_ops[i][4]
                    if so not in first_for_so:
                        first_for_so[so] = i
                    last_for_so[so] = i

                acc = {}
                with tc.tile_pool(
                    name=f"acc{sos[0]}", bufs=8, space="PSUM"
                ) as acc_pool:
                    for so in sos:
                        if so in first_for_so:
                            acc[so] = acc_pool.tile(
                                [U, ZS],
                                mybir.dt.float32,
                                tag=f"acc{sos.index(so)}",
                                name=f"acc_{so}",
                                bufs=1,
                            )

                    def rslot(r):
                        if r[0] in ("sg", "pair"):
                            return ("prod", slot_of[r])
                        if r[0] == "x1":
                            return ("x1t", r[1])
                        return ("x0g", r[1])

                    def strided(region, lo, step, n):
                        t = {"x1t": x1t, "x0g": x0g, "prod": prod}[region]
                        base = t[:]
                        pitch = base.ap[0][0]
                        return _AP(
                            base.tensor,
                            base.offset + lo * ZS,
                            [[pitch, U], [step * ZS, n], [1, ZS]],
                        )

                    def resolve(region, idx):
                        t = {"x1t": x1t, "x0g": x0g, "prod": prod}[region]
                        return seg(t, idx)

                    def _ensure_slab(sj):
                        tiles = slab_state.setdefault("tiles", {})
                        if sj in tiles or sj >= n_slabs:
                            return
                        st = slab_pool.tile(
                            [U, SLAB * U], dt, tag="slab", name=f"slab{sj}"
                        )
                        tiles[sj] = st
                        nc.sync.dma_start(
                            out=st[:].rearrange("p (d c) -> p d c", d=SLAB),
                            in_=cd_d[sj * SLAB * U : (sj + 1) * SLAB * U, :]
                            .rearrange("(d p) c -> p d c", p=U),
                        )

                    def get_slab(gidx):
                        sj = gidx // SLAB
                        _ensure_slab(sj)
                        _ensure_slab(sj + 1)
                        return slab_state["tiles"][sj]

                    mm_fifo = []
                    depth_state = [pipe_depth]

                    def _evac(so):
                        ev_t = evac_pool.tile(
                            [U, ZS], dt, tag="ev", name=f"ev{so}"
                        )
                        nc.scalar.copy(out=ev_t[:], in_=acc[so][:])
                        nc.sync.dma_start(
                            out=out_d[so * U : (so + 1) * U, :], in_=ev_t[:]
                        )

                    def _mm_now(i, rhs_ap):
                        d, r1, r2, c, so = path_ops[i]
                        gidx = len(coeff_order)
                        coeff_order.append(c)
                        st = get_slab(gidx)
                        sk = gidx % SLAB
                        nc.tensor.matmul(
                            acc[so][:],
                            st[:, sk * U : (sk + 1) * U],
                            rhs_ap,
                            start=(i == first_for_so[so]),
                            stop=(i == last_for_so[so]),
                        )
                        if i == last_for_so[so]:
                            _evac(so)

                    def emit_mm(i, rhs_ap):
                        mm_fifo.append((i, rhs_ap))
                        while len(mm_fifo) > depth_state[0]:
                            _mm_now(*mm_fifo.pop(0))

                    def flush_mms():
                        while mm_fifo:
                            _mm_now(*mm_fifo.pop(0))

                    n_events = len(events)
                    for ev_idx, ev in enumerate(events):
                        if gi == len(groups) - 1 and ev_idx >= n_events - 6:
                            depth_state[0] = max(
                                0, (n_events - 1 - ev_idx) * 3
                            )
                        kind = ev[0]
                        if kind == "build_sg":
                            _, s0, s_lo, n, slot_lo = ev
                            in0 = (
                                seg(x0g, s0)
                                .rearrange("p (o z) -> p o z", o=1)
                                .broadcast_to([U, n, ZS])
                            )
                            in1 = span(x1t, s_lo, n).rearrange(
                                "p (r z) -> p r z", r=n
                            )
                            out = span(prod, slot_lo, n).rearrange(
                                "p (r z) -> p r z", r=n
                            )
                            nc.vector.tensor_tensor(
                                out=out, in0=in0, in1=in1, op=MULT
                            )
                        elif kind == "build_pair_run":
                            _, delta, a_lo, n, slot_lo = ev
                            in0 = span(x1t, a_lo, n).rearrange(
                                "p (r z) -> p r z", r=n
                            )
                            in1 = span(x1t, a_lo + delta, n).rearrange(
                                "p (r z) -> p r z", r=n
                            )
                            out = span(prod, slot_lo, n).rearrange(
                                "p (r z) -> p r z", r=n
                            )
                            nc.vector.tensor_tensor(
                                out=out, in0=in0, in1=in1, op=MULT
                            )
                        elif kind == "mm":
                            i = ev[1]
                            emit_mm(i, resolve(*rslot(path_ops[i][1])))
                        else:  # finals
                            chunk = ev[1]
                            n = len(chunk)
                            rt = ring_pool.tile(
                                [U, ring_w * ZS],
                                dt,
                                tag="ring",
                                name=f"rg{gi}_{chunk[0]}",
                            )
                            refs1 = [rslot(path_ops[i][1]) for i in chunk]
                            refs2 = [rslot(path_ops[i][2]) for i in chunk]
                            if n == 1:
                                nc.vector.tensor_tensor(
                                    out=rt[:, 0:ZS],
                                    in0=resolve(*refs1[0]),
                                    in1=resolve(*refs2[0]),
                                    op=MULT,
                                )
                            else:
                                st1 = refs1[1][1] - refs1[0][1]
                                st2 = refs2[1][1] - refs2[0][1]
                                in0 = strided(refs1[0][0], refs1[0][1], st1, n)
                                in1 = strided(refs2[0][0], refs2[0][1], st2, n)
                                out = span(rt, 0, n).rearrange(
                                    "p (r z) -> p r z", r=n
                                )
                                nc.vector.tensor_tensor(
                                    out=out, in0=in0, in1=in1, op=MULT
                                )
                            for k, i in enumerate(chunk):
                                emit_mm(i, rt[:, k * ZS : (k + 1) * ZS])

                    flush_mms()
                    for so in sos:
                        if so not in acc:
                            ev_t = evac_pool.tile(
                                [U, ZS], dt, tag="ev", name=f"ev{so}"
                            )
                            nc.vector.memset(ev_t[:], 0.0)
                            nc.sync.dma_start(
                                out=out_d[so * U : (so + 1) * U, :], in_=ev_t[:]
                            )

    nc.compile()
    return nc, coeff_order


def kernel(x0, x1, coeff1, coeff2, coeff3, i0, idx1, idx2, idx3):
    global LAST_EXEC_NS, LAST_RESULTS
    from concourse.bass_utils import run_bass_kernel_spmd
    import ml_dtypes

    npdt = ml_dtypes.bfloat16

    x0 = np.asarray(x0, dtype=np.float32)
    x1 = np.asarray(x1, dtype=np.float32)
    i0 = np.asarray(i0).astype(np.int64)
    idxs = [np.asarray(a) for a in (idx1, idx2, idx3)]
    coeffs = [np.asarray(c, dtype=np.float32) for c in (coeff1, coeff2, coeff3)]

    pool_cols = int(os.environ.get("KERNEL_POOL_COLS", "0"))
    ring_w = int(os.environ.get("KERNEL_RING_W", "5"))
    pipe_depth = int(os.environ.get("KERNEL_PIPE_DEPTH", "16"))

    groups, all_sq, _paths, _forms = _build_plan(idxs, coeffs)
    nc, coeff_order = _build_bass(groups, all_sq, None, ring_w, pipe_depth)

    n_slabs = (len(coeff_order) + SLAB - 1) // SLAB
    cdiag = np.zeros((n_slabs * SLAB * U, U), dtype=npdt)
    for gidx, c in enumerate(coeff_order):
        blk = cdiag[gidx * U : (gidx + 1) * U, :]
        np.fill_diagonal(blk, np.asarray(c, dtype=npdt))

    in_maps = []
    eye = np.arange(NELEM)
    x0c = x0.astype(npdt)
    for c in range(NCORES):
        zl, zh = c * ZS, (c + 1) * ZS
        shard = x1[zl:zh]
        x1t = np.ascontiguousarray(
            shard.reshape(ZS, S, U).transpose(1, 2, 0).reshape(S * U, ZS)
        ).astype(npdt)
        oh = (i0[zl:zh][None, :] == eye[:, None]).astype(npdt)
        in_maps.append({"x1t": x1t, "x0w": x0c, "oh": oh, "cdiag": cdiag})

    trace = os.environ.get("BASS_TRACE", "") not in ("", "0")
    trace_cores = None
    tc_env = os.environ.get("KERNEL_TRACE_CORES", "")
    if tc_env:
        trace_cores = [int(x) for x in tc_env.split(",")]
    res = run_bass_kernel_spmd(
        nc,
        in_maps,
        core_ids=list(range(NCORES)),
        trace=trace,
        trace_cores=trace_cores,
    )
    LAST_EXEC_NS = res.exec_time_ns
    LAST_RESULTS = res

    out = np.empty((Z, S * U), dtype=np.float32)
    for c in range(NCORES):
        outt = np.asarray(res.results[c]["outt"], dtype=np.float32)
        out[c * ZS : (c + 1) * ZS] = (
            outt.reshape(S, U, ZS).transpose(2, 0, 1).reshape(ZS, S * U)
        )
    return out

